# revision 4
# baseline (speedup 1.0000x reference)
"""Trainium2 Bass kernel for nn_DecoderAttention (bilinear-score attention).

Computes, for full inputs h_d_t [32,1024], h_d_all [32,4096,1024], W [1024,1024]:
    qW    = h_d_t @ W
    e     = einsum('bd,btd->bt', qW, h_d_all)
    alpha = exp(e) / (sum(e, axis=1) + 1e-8)
    c_t   = einsum('bt,btd->bd', alpha, h_d_all)

Strategy: data-parallel over batch — 4 batches per NeuronCore across 8 cores,
W replicated. Single pass over the cache: per 128-row t-tile, a fused DVE
multiply+reduce produces the scores, ACT exponentiates, and the TensorEngine
accumulates exp(e)^T @ h into PSUM (float32r fast path). The denominator is
the raw-score running sum, applied at the end, so h_d_all is read exactly once.
"""

import numpy as np

import concourse.bass as bass  # noqa: F401  (engine types pulled via bacc)
import concourse.mybir as mybir
import concourse.tile as tile
from concourse import bacc, bass_utils

B, T, D = 32, 4096, 1024
N_CORES = 8
B_LOC = B // N_CORES  # 4 batches per core
TT = 128              # t-tile rows (matmul contraction dim)
NT = T // TT          # 32 tiles per batch
EPS = 1e-8

_NC_CACHE = {}


def _build_module():
    f32 = mybir.dt.float32
    f32r = mybir.dt.float32r

    nc = bacc.Bacc("TRN2", debug=False, num_devices=N_CORES)
    h_d = nc.dram_tensor("h", [B_LOC, T, D], f32, kind="ExternalInput")
    hdtT_d = nc.dram_tensor("hdtT", [D, B_LOC], f32, kind="ExternalInput")
    w_d = nc.dram_tensor("W", [D, D], f32, kind="ExternalInput")
    c_d = nc.dram_tensor("c", [B_LOC, D], f32, kind="ExternalOutput")

    h_ap = h_d.ap().rearrange("b (n p) d -> b n p d", p=TT)      # [4, 32, 128, 1024]
    w_ap = w_d.ap().rearrange("(c p) j -> c p j", p=128)         # [8, 128, 1024]
    hdtT_ap = hdtT_d.ap().rearrange("(c p) b -> c p b", p=128)   # [8, 128, 4]

    with tile.TileContext(nc) as tc:
        with (
            tc.tile_pool(name="wpool", bufs=1) as wpool,
            tc.tile_pool(name="qpool", bufs=1) as qpool,
            tc.tile_pool(name="hpool", bufs=6) as hpool,
            tc.tile_pool(name="spool", bufs=2) as spool,
            tc.tile_pool(name="ppool", bufs=4) as ppool,
            tc.tile_pool(name="epool", bufs=2) as epool,
            tc.tile_pool(name="fpool", bufs=2) as fpool,
            tc.tile_pool(name="psq", bufs=1, space="PSUM") as psq,
            tc.tile_pool(name="psn", bufs=2, space="PSUM") as psn,
            tc.tile_pool(name="psd", bufs=1, space="PSUM") as psd,
        ):
            # ---- qW = h_d_t @ W for the local batches ----
            w_sb = wpool.tile([128, 8 * D], f32)
            hdt_sb = wpool.tile([128, 8 * B_LOC], f32)
            for c in range(8):
                nc.sync.dma_start(w_sb[:, c * D:(c + 1) * D], w_ap[c])
                nc.sync.dma_start(hdt_sb[:, c * B_LOC:(c + 1) * B_LOC], hdtT_ap[c])
            qw_ps = psq.tile([B_LOC, D], f32)
            for c in range(8):
                for j in range(2):
                    nc.tensor.matmul(
                        qw_ps[:, j * 512:(j + 1) * 512],
                        hdt_sb[:, c * B_LOC:(c + 1) * B_LOC],
                        w_sb[:, c * D + j * 512: c * D + (j + 1) * 512],
                        start=(c == 0),
                        stop=(c == 7),
                    )
            qw_sb = qpool.tile([B_LOC, D], f32)
            nc.scalar.copy(qw_sb[:], qw_ps[:])
            # replicate each batch's qW row across all 128 partitions
            # (partition_broadcast needs its source at partition 0, so stage
            # each row through a partition-0 tile via SBUF->SBUF DMA first)
            qwb = qpool.tile([128, B_LOC * D], f32)
            for b in range(B_LOC):
                qrow = ppool.tile([1, D], f32, tag="qrow")
                nc.sync.dma_start(qrow[:], qw_sb[b:b + 1, :])
                nc.gpsimd.partition_broadcast(qwb[:, b * D:(b + 1) * D], qrow[:])
            ones_sb = qpool.tile([128, 1], f32)
            nc.vector.memset(ones_sb[:], 1.0)

            # ---- main single pass over the cache ----
            for b in range(B_LOC):
                e_b = epool.tile([128, NT], f32)
                num_ps = psn.tile([1, D], f32)
                for i in range(NT):
                    h_t = hpool.tile([TT, D], f32r)
                    nc.sync.dma_start(h_t[:], h_ap[b, i].bitcast(f32r))
                    prod = spool.tile([TT, D], f32)
                    # fused multiply+reduce: prod = h*qW, e_b[:,i] = sum(prod)
                    # (tensor_tensor_reduce crashes this runtime; the
                    # scalar_tensor_tensor opcode with accum_out is equivalent)
                    nc.vector.scalar_tensor_tensor(
                        out=prod[:],
                        in0=h_t[:].bitcast(f32),
                        scalar=1.0,
                        in1=qwb[:, b * D:(b + 1) * D],
                        op0=mybir.AluOpType.mult,
                        op1=mybir.AluOpType.mult,
                        accum_out=e_b[:, i:i + 1],
                    )
                    p_t = ppool.tile([TT, 1], f32r)
                    nc.scalar.activation(
                        p_t[:], e_b[:, i:i + 1], mybir.ActivationFunctionType.Exp
                    )
                    for j in range(2):
                        nc.tensor.matmul(
                            num_ps[:, j * 512:(j + 1) * 512],
                            p_t[:],
                            h_t[:, j * 512:(j + 1) * 512],
                            start=(i == 0),
                            stop=(i == NT - 1),
                        )
                # ---- finalize batch b ----
                e_red = fpool.tile([128, 1], f32)
                nc.vector.tensor_reduce(
                    e_red[:], e_b[:], axis=mybir.AxisListType.X, op=mybir.AluOpType.add
                )
                den_ps = psd.tile([1, 1], f32)
                nc.tensor.matmul(den_ps[:], e_red[:], ones_sb[:], start=True, stop=True)
                den_sb = fpool.tile([1, 1], f32)
                nc.vector.tensor_scalar_add(den_sb[:], den_ps[:], EPS)
                recip = fpool.tile([1, 1], f32)
                nc.vector.reciprocal(recip[:], den_sb[:])
                c_sb = fpool.tile([1, D], f32)
                nc.vector.tensor_scalar_mul(c_sb[:], num_ps[:], recip[:])
                nc.sync.dma_start(c_d.ap()[b:b + 1, :], c_sb[:])

    nc.compile()
    return nc


def _get_module():
    if "nc" not in _NC_CACHE:
        _NC_CACHE["nc"] = _build_module()
    return _NC_CACHE["nc"]


def _make_in_maps(h_d_t, h_d_all, W):
    h_d_t = np.ascontiguousarray(np.asarray(h_d_t), dtype=np.float32)
    h_d_all = np.ascontiguousarray(np.asarray(h_d_all), dtype=np.float32)
    W = np.ascontiguousarray(np.asarray(W), dtype=np.float32)
    in_maps = []
    for i in range(N_CORES):
        sl = slice(i * B_LOC, (i + 1) * B_LOC)
        in_maps.append(
            {
                "h": h_d_all[sl],
                "hdtT": np.ascontiguousarray(h_d_t[sl].T),
                "W": W,
            }
        )
    return in_maps


def kernel(h_d_t, h_d_all, W, **run_kwargs):
    nc = _get_module()
    in_maps = _make_in_maps(h_d_t, h_d_all, W)
    res = bass_utils.run_bass_kernel_spmd(
        nc, in_maps, core_ids=list(range(N_CORES)), **run_kwargs
    )
    out = np.concatenate([res.results[i]["c"] for i in range(N_CORES)], axis=0)
    if run_kwargs:
        kernel.last_results = res
    return out


# revision 7
# speedup vs baseline: 19866.4290x; 19866.4290x over previous
"""Trainium2 Bass kernel for nn_DecoderAttention (bilinear-score attention).

Computes, for full inputs h_d_t [32,1024], h_d_all [32,4096,1024], W [1024,1024]:
    qW    = h_d_t @ W
    e     = einsum('bd,btd->bt', qW, h_d_all)
    alpha = exp(e) / (sum(e, axis=1) + 1e-8)
    c_t   = einsum('bt,btd->bd', alpha, h_d_all)

Strategy: data-parallel over batch — 4 batches per NeuronCore across 8 cores,
W replicated. Single pass over the cache: per 128-row t-tile, a fused DVE
multiply+reduce produces the scores, ACT exponentiates, and the TensorEngine
accumulates exp(e)^T @ h into PSUM (float32r fast path). The denominator is
the raw-score running sum, applied at the end, so h_d_all is read exactly once.
"""

import numpy as np

import concourse.bass as bass  # noqa: F401  (engine types pulled via bacc)
import concourse.mybir as mybir
import concourse.tile as tile
from concourse import bacc, bass_utils

B, T, D = 32, 4096, 1024
N_CORES = 8
B_LOC = B // N_CORES  # 4 batches per core
TT = 128              # t-tile rows (matmul contraction dim)
NT = T // TT          # 32 tiles per batch
EPS = 1e-8
MM_MODE = "f32r"  # "f32r": fast PE path + ACT-side rounding copy; "f32": exact PE

_NC_CACHE = {}


def _build_module():
    f32 = mybir.dt.float32
    f32r = mybir.dt.float32r
    MM_DT = f32r if MM_MODE == "f32r" else f32

    nc = bacc.Bacc("TRN2", debug=False, num_devices=N_CORES)
    h_d = nc.dram_tensor("h", [B_LOC, T, D], f32, kind="ExternalInput")
    hdtT_d = nc.dram_tensor("hdtT", [D, B_LOC], f32, kind="ExternalInput")
    w_d = nc.dram_tensor("W", [D, D], f32, kind="ExternalInput")
    c_d = nc.dram_tensor("c", [B_LOC, D], f32, kind="ExternalOutput")

    h_ap = h_d.ap().rearrange("b (n p) d -> b n p d", p=TT)      # [4, 32, 128, 1024]
    w_ap = w_d.ap().rearrange("(c p) j -> c p j", p=128)         # [8, 128, 1024]
    hdtT_ap = hdtT_d.ap().rearrange("(c p) b -> c p b", p=128)   # [8, 128, 4]

    with tile.TileContext(nc) as tc:
        with (
            tc.tile_pool(name="wpool", bufs=1) as wpool,
            tc.tile_pool(name="qpool", bufs=1) as qpool,
            tc.tile_pool(name="hpool", bufs=6) as hpool,
            tc.tile_pool(name="spool", bufs=2) as spool,
            tc.tile_pool(name="ppool", bufs=4) as ppool,
            tc.tile_pool(name="epool", bufs=2) as epool,
            tc.tile_pool(name="fpool", bufs=2) as fpool,
            tc.tile_pool(name="psq", bufs=1, space="PSUM") as psq,
            tc.tile_pool(name="psn", bufs=2, space="PSUM") as psn,
            tc.tile_pool(name="psd", bufs=1, space="PSUM") as psd,
        ):
            # ---- qW = h_d_t @ W for the local batches ----
            w_sb = wpool.tile([128, 8 * D], f32)
            hdt_sb = wpool.tile([128, 8 * B_LOC], f32)
            for c in range(8):
                nc.sync.dma_start(w_sb[:, c * D:(c + 1) * D], w_ap[c])
                nc.sync.dma_start(hdt_sb[:, c * B_LOC:(c + 1) * B_LOC], hdtT_ap[c])
            qw_ps = psq.tile([B_LOC, D], f32)
            for c in range(8):
                for j in range(2):
                    nc.tensor.matmul(
                        qw_ps[:, j * 512:(j + 1) * 512],
                        hdt_sb[:, c * B_LOC:(c + 1) * B_LOC],
                        w_sb[:, c * D + j * 512: c * D + (j + 1) * 512],
                        start=(c == 0),
                        stop=(c == 7),
                    )
            qw_sb = qpool.tile([B_LOC, D], f32)
            nc.scalar.copy(qw_sb[:], qw_ps[:])
            # replicate each batch's qW row across all 128 partitions
            # (partition_broadcast needs its source at partition 0, so stage
            # each row through a partition-0 tile via SBUF->SBUF DMA first)
            qwb = qpool.tile([128, B_LOC * D], f32)
            for b in range(B_LOC):
                qrow = ppool.tile([1, D], f32, tag="qrow")
                nc.sync.dma_start(qrow[:], qw_sb[b:b + 1, :])
                nc.gpsimd.partition_broadcast(qwb[:, b * D:(b + 1) * D], qrow[:])
            ones_sb = qpool.tile([128, 1], f32)
            nc.vector.memset(ones_sb[:], 1.0)

            # ---- main single pass over the cache ----
            for b in range(B_LOC):
                e_b = epool.tile([128, NT], f32)
                num_ps = psn.tile([1, D], f32)
                for i in range(NT):
                    h_t = hpool.tile([TT, D], f32)
                    nc.sync.dma_start(h_t[:], h_ap[b, i])
                    prod = spool.tile([TT, D], f32)
                    # fused multiply+reduce: prod = h*qW, e_b[:,i] = sum(prod)
                    # (tensor_tensor_reduce crashes this runtime; the
                    # scalar_tensor_tensor opcode with accum_out is equivalent)
                    nc.vector.scalar_tensor_tensor(
                        out=prod[:],
                        in0=h_t[:],
                        scalar=1.0,
                        in1=qwb[:, b * D:(b + 1) * D],
                        op0=mybir.AluOpType.mult,
                        op1=mybir.AluOpType.mult,
                        accum_out=e_b[:, i:i + 1],
                    )
                    p_t = ppool.tile([TT, 1], MM_DT)
                    nc.scalar.activation(
                        p_t[:], e_b[:, i:i + 1], mybir.ActivationFunctionType.Exp
                    )
                    if MM_DT == f32r:
                        # scores need full-precision h (the denominator nearly
                        # cancels), so the f32r copy for the PE is made on the
                        # otherwise-idle ScalarEngine rather than rounding in
                        # the DMA.
                        h_mm = spool.tile([TT, D], f32r, tag="hmm")
                        nc.scalar.copy(h_mm[:], h_t[:])
                    else:
                        h_mm = h_t
                    for j in range(2):
                        nc.tensor.matmul(
                            num_ps[:, j * 512:(j + 1) * 512],
                            p_t[:],
                            h_mm[:, j * 512:(j + 1) * 512],
                            start=(i == 0),
                            stop=(i == NT - 1),
                        )
                # ---- finalize batch b ----
                e_red = fpool.tile([128, 1], f32)
                nc.vector.tensor_reduce(
                    e_red[:], e_b[:], axis=mybir.AxisListType.X, op=mybir.AluOpType.add
                )
                den_ps = psd.tile([1, 1], f32)
                nc.tensor.matmul(den_ps[:], e_red[:], ones_sb[:], start=True, stop=True)
                den_sb = fpool.tile([1, 1], f32)
                nc.vector.tensor_scalar_add(den_sb[:], den_ps[:], EPS)
                recip = fpool.tile([1, 1], f32)
                nc.vector.reciprocal(recip[:], den_sb[:])
                c_sb = fpool.tile([1, D], f32)
                nc.vector.tensor_scalar_mul(c_sb[:], num_ps[:], recip[:])
                nc.sync.dma_start(c_d.ap()[b:b + 1, :], c_sb[:])

    nc.compile()
    return nc


def _get_module():
    if "nc" not in _NC_CACHE:
        _NC_CACHE["nc"] = _build_module()
    return _NC_CACHE["nc"]


def _make_in_maps(h_d_t, h_d_all, W):
    h_d_t = np.ascontiguousarray(np.asarray(h_d_t), dtype=np.float32)
    h_d_all = np.ascontiguousarray(np.asarray(h_d_all), dtype=np.float32)
    W = np.ascontiguousarray(np.asarray(W), dtype=np.float32)
    in_maps = []
    for i in range(N_CORES):
        sl = slice(i * B_LOC, (i + 1) * B_LOC)
        in_maps.append(
            {
                "h": h_d_all[sl],
                "hdtT": np.ascontiguousarray(h_d_t[sl].T),
                "W": W,
            }
        )
    return in_maps


def kernel(h_d_t, h_d_all, W, **run_kwargs):
    nc = _get_module()
    in_maps = _make_in_maps(h_d_t, h_d_all, W)
    res = bass_utils.run_bass_kernel_spmd(
        nc, in_maps, core_ids=list(range(N_CORES)), **run_kwargs
    )
    out = np.concatenate([res.results[i]["c"] for i in range(N_CORES)], axis=0)
    if run_kwargs:
        kernel.last_results = res
    return out
